# revision 5
# baseline (speedup 1.0000x reference)
"""Margin-based triplet loss (nn_Criterion) for Trainium2, 8 NeuronCores.

Strategy: anchor-block sharding + margin-slack reduction.  Core c owns
anchor rows [512c, 512c+512).  The host buckets triplets by anchor block
into dense pair-count histograms (wp for anchor-positive pairs, wc =
wp + wn for the detector); the device computes squared distances densely
via fp8 DoubleRow matmuls (with the norm terms folded in as augmented
bf16 contraction rows), takes one sqrt per tile, and reduces just TWO
dense quantities per anchor row:

    wd  = sum_b wp[a,b] * d[a,b]          (weighted distance sum)
    det = sum_b wc[a,b] * [d[a,b] < cneg] (margin-violation detector)

If det == 0 everywhere (certified on-device), the exact loss reduces to

    pos_sum = wd_total - sum_a cpos(a) * wp_row(a)     (host, f64)
    neg_sum = 0,  pair_count = total wp count
    (+ exact host corrections for degenerate a==p / a==n triplets)

because every weighted pair then has d > cneg > cpos, making every
pos relu affine-active and every neg relu inactive.  If det != 0 the
kernel falls back to an exact host computation (never triggers for
Gaussian-scale data; distances concentrate near sqrt(2D) >> beta+margin).
"""

import os

import numpy as np

B, D, T, NCLS = 4096, 1024, 65536, 100
MARGIN = 0.2
EPS = 1e-8
NCORES = 8
P = 128
RB = 512                 # anchor rows per core
NRB = RB // P            # 4 anchor row blocks
KCH = D // P             # 8 contraction chunks of 128
KP = KCH // 2            # 4 chunk pairs (DoubleRow)
GCOLS = 2048             # columns per join group
NJJ = B // GCOLS         # 2 column groups
NG = NRB * NJJ           # 8 groups per core
D2_BIAS = 0.0625         # keeps diagonal d^2 strictly positive for sqrt

_COMPILED = None
LAST_RESULTS = None


def _build_nc():
    import concourse.bacc as bacc
    import concourse.bass as bass
    import concourse.mybir as mybir
    import concourse.tile as tile

    f32 = mybir.dt.float32
    bf16 = mybir.dt.bfloat16
    f8 = mybir.dt.float8e4
    Alu = mybir.AluOpType
    Act = mybir.ActivationFunctionType

    use_dr = not bool(int(os.environ.get("KERNEL_NO_DR", "0")))
    dr = mybir.MatmulPerfMode.DoubleRow if use_dr else None

    nc = bacc.Bacc("TRN2")

    xt_d = nc.dram_tensor("xt", [P, KCH, B], f8, kind="ExternalInput")
    xat_d = nc.dram_tensor("xat", [P, KCH, RB], f8, kind="ExternalInput")
    wp_d = nc.dram_tensor("wp", [P, NRB, B], bf16, kind="ExternalInput")
    wc_d = nc.dram_tensor("wc", [P, NRB, B], bf16, kind="ExternalInput")
    augs_d = nc.dram_tensor("augs", [4, NRB, P], bf16, kind="ExternalInput")
    augm_d = nc.dram_tensor("augm", [4, B], bf16, kind="ExternalInput")
    cneg_d = nc.dram_tensor("cneg", [P, NRB], f32, kind="ExternalInput")
    out_d = nc.dram_tensor("out", [P, 16], f32, kind="ExternalOutput")

    with tile.TileContext(nc) as tc:
        with (
            tc.tile_pool(name="const", bufs=1) as constp,
            tc.tile_pool(name="xt", bufs=1) as xtp,
            tc.tile_pool(name="w", bufs=3) as wpool,
            tc.tile_pool(name="d", bufs=2) as dpool,
            tc.tile_pool(name="ps", bufs=2, space="PSUM") as gpsum,
        ):
            xat_sb = constp.tile([P, KCH, RB], f8, tag="xat")
            augs_sb = constp.tile([4, NRB, P], bf16, tag="augs")
            augm_sb = constp.tile([4, B], bf16, tag="augm")
            cneg_sb = constp.tile([P, NRB], f32, tag="cneg")
            acc = constp.tile([P, 16], f32, tag="acc")
            # small tiles first so the aug matmuls can start ASAP
            nc.sync.dma_start(augs_sb[:], augs_d[:])
            nc.sync.dma_start(augm_sb[:], augm_d[:])
            nc.sync.dma_start(cneg_sb[:], cneg_d[:])
            nc.sync.dma_start(xat_sb[:], xat_d[:])

            xtp_sb = []
            for i2 in range(KP):
                t = xtp.tile([P, 2, B], f8, tag=f"xt{i2}", name=f"xt{i2}")
                nc.sync.dma_start(t[:], xt_d[:, 2 * i2 : 2 * i2 + 2, :])
                xtp_sb.append(t)

            for r in range(NRB):
                for jj in range(NJJ):
                    gi = r * NJJ + jj
                    c0 = jj * GCOLS
                    wpt = wpool.tile([P, GCOLS], bf16, tag="wp")
                    wct = wpool.tile([P, GCOLS], bf16, tag="wc")
                    nc.scalar.dma_start(wpt[:], wp_d[:, r, c0 : c0 + GCOLS])
                    nc.scalar.dma_start(wct[:], wc_d[:, r, c0 : c0 + GCOLS])

                    g = gpsum.tile([P, GCOLS], f32, tag="g", space="PSUM")
                    # norm terms first (k=4 aug rows: na_hi, na_lo, 1, 1)
                    for j in range(GCOLS // 512):
                        cs = slice(c0 + j * 512, c0 + (j + 1) * 512)
                        nc.tensor.matmul(
                            g[:, j * 512 : (j + 1) * 512],
                            augs_sb[:, r, :],
                            augm_sb[:, cs],
                            start=True,
                            stop=False,
                        )
                    # -2 * Gram, fp8 (DoubleRow: two 128-chunks per pass)
                    if use_dr:
                        for i2 in range(KP):
                            lhs = xat_sb[:, 2 * i2 : 2 * i2 + 2, r * P : (r + 1) * P]
                            for j in range(GCOLS // 512):
                                cs = slice(c0 + j * 512, c0 + (j + 1) * 512)
                                nc.tensor.matmul(
                                    g[:, j * 512 : (j + 1) * 512],
                                    lhs,
                                    xtp_sb[i2][:, :, cs],
                                    start=False,
                                    stop=(i2 == KP - 1),
                                    perf_mode=dr,
                                )
                    else:
                        for i in range(KCH):
                            lhs = xat_sb[:, i, r * P : (r + 1) * P]
                            for j in range(GCOLS // 512):
                                cs = slice(c0 + j * 512, c0 + (j + 1) * 512)
                                nc.tensor.matmul(
                                    g[:, j * 512 : (j + 1) * 512],
                                    lhs,
                                    xtp_sb[i // 2][:, i % 2, cs],
                                    start=False,
                                    stop=(i == KCH - 1),
                                )

                    d = dpool.tile([P, GCOLS], bf16, tag="d")
                    nc.scalar.activation(d[:], g[:], Act.Sqrt)
                    dum_a = dpool.tile([P, GCOLS], bf16, tag="dum_a")
                    dum_b = dpool.tile([P, GCOLS], bf16, tag="dum_b")
                    nc.vector.scalar_tensor_tensor(
                        dum_a[:], d[:], 1.0, wpt[:],
                        Alu.mult, Alu.mult,
                        accum_out=acc[:, gi : gi + 1],
                    )
                    nc.vector.scalar_tensor_tensor(
                        dum_b[:], d[:], cneg_sb[:, r : r + 1], wct[:],
                        Alu.is_lt, Alu.mult,
                        accum_out=acc[:, 8 + gi : 8 + gi + 1],
                    )

            nc.sync.dma_start(out_d[:], acc[:])

    nc.compile()
    return nc


def _prep_inputs(batch, labels, triplets, beta):
    import ml_dtypes

    bf = ml_dtypes.bfloat16
    f8 = ml_dtypes.float8_e4m3
    trip = np.asarray(triplets).astype(np.int64)
    labs = np.asarray(labels).astype(np.int64)
    batch = np.ascontiguousarray(np.asarray(batch), dtype=np.float32)
    beta64 = np.asarray(beta).astype(np.float64)

    x8 = batch.astype(f8)
    x8f = x8.astype(np.float32)
    # norms of the fp8-rounded rows (consistency: d^2 = |a-b|^2 >= 0)
    n64 = (x8f.astype(np.float64) ** 2).sum(-1)
    n32 = n64.astype(np.float32)
    n_hi = n32.astype(bf)
    n_hi_f = n_hi.astype(np.float32)
    n_lo = (n32 - n_hi_f + np.float32(D2_BIAS)).astype(bf)

    # X^T chunked: xt[p, i, b] = x8[b, i*128+p]
    xt = np.ascontiguousarray(x8.reshape(B, KCH, P).transpose(2, 1, 0))
    # moving aug rows: [1, 1, nb_hi, nb_lo]
    augm = np.empty((4, B), dtype=bf)
    augm[0, :] = bf(1.0)
    augm[1, :] = bf(1.0)
    augm[2, :] = n_hi
    augm[3, :] = n_lo

    cpos_all = beta64[labs] - MARGIN          # [B] per-anchor pos threshold
    cneg_all = (beta64[labs] + MARGIN).astype(np.float32)

    t0, t1, t2 = trip[:, 0], trip[:, 1], trip[:, 2]
    deg_p = t0 == t1
    deg_n = t0 == t2

    # exact host corrections for degenerate triplets (d = sqrt(EPS))
    d_deg = np.sqrt(EPS)
    host_sum = 0.0
    host_cnt = 0.0
    if deg_p.any():
        pl = np.maximum(d_deg - beta64[labs[t0[deg_p]]] + MARGIN, 0.0)
        host_sum += pl.sum()
        host_cnt += (pl > 0).sum()
    if deg_n.any():
        nl = np.maximum(beta64[labs[t0[deg_n]]] + MARGIN - d_deg, 0.0)
        host_sum += nl.sum()
        host_cnt += (nl > 0).sum()

    in_maps = []
    host_terms = {"sum": host_sum, "cnt": host_cnt, "cposw": 0.0, "wptot": 0.0}
    xm2_all = (-2.0 * x8f)  # exact scaling of fp8 values
    for c in range(NCORES):
        lo, hi = c * RB, (c + 1) * RB
        sel = (t0 >= lo) & (t0 < hi)
        sp = sel & ~deg_p
        sn = sel & ~deg_n
        a_p = t0[sp] - lo
        a_n = t0[sn] - lo
        wp = np.bincount(a_p * B + t1[sp], minlength=RB * B).reshape(RB, B)
        wn = np.bincount(a_n * B + t2[sn], minlength=RB * B).reshape(RB, B)
        wc = wp + wn

        host_terms["cposw"] += float((cpos_all[lo:hi] * wp.sum(-1)).sum())
        host_terms["wptot"] += float(wp.sum())

        def togrid(w):
            return np.ascontiguousarray(
                w.reshape(NRB, P, B).transpose(1, 0, 2)
            ).astype(bf)

        xat = np.ascontiguousarray(
            xm2_all[lo:hi].astype(f8).reshape(RB, KCH, P).transpose(2, 1, 0)
        )
        augs = np.empty((4, NRB, P), dtype=bf)
        augs[0] = n_hi[lo:hi].reshape(NRB, P)
        augs[1] = n_lo[lo:hi].reshape(NRB, P)
        augs[2] = bf(1.0)
        augs[3] = bf(1.0)
        cneg = np.ascontiguousarray(cneg_all[lo:hi].reshape(NRB, P).T)

        in_maps.append(
            {
                "xt": xt,
                "xat": xat,
                "wp": togrid(wp),
                "wc": togrid(wc),
                "augs": augs,
                "augm": augm,
                "cneg": cneg,
            }
        )
    return in_maps, host_terms


def _host_reference(batch, labels, triplets, beta):
    """Exact fallback (f64 accumulation), chunked to bound memory."""
    batch = np.asarray(batch, dtype=np.float32)
    trip = np.asarray(triplets).astype(np.int64)
    labs = np.asarray(labels).astype(np.int64)
    beta64 = np.asarray(beta).astype(np.float64)
    total = 0.0
    cnt = 0.0
    for s in range(0, trip.shape[0], 8192):
        t = trip[s : s + 8192]
        a = batch[t[:, 0]].astype(np.float64)
        p = batch[t[:, 1]].astype(np.float64)
        n = batch[t[:, 2]].astype(np.float64)
        d_ap = np.sqrt(((a - p) ** 2).sum(-1) + EPS)
        d_an = np.sqrt(((a - n) ** 2).sum(-1) + EPS)
        bt = beta64[labs[t[:, 0]]]
        pos = np.maximum(d_ap - bt + MARGIN, 0.0)
        neg = np.maximum(bt - d_an + MARGIN, 0.0)
        total += pos.sum() + neg.sum()
        cnt += (pos > 0).sum() + (neg > 0).sum()
    return np.float32(total if cnt == 0.0 else total / cnt)


def kernel(batch, labels, triplets, beta):
    global _COMPILED, LAST_RESULTS
    from concourse.bass_utils import run_bass_kernel_spmd

    if _COMPILED is None:
        _COMPILED = _build_nc()
    nc = _COMPILED

    in_maps, ht = _prep_inputs(batch, labels, triplets, beta)
    trace = bool(int(os.environ.get("KERNEL_TRACE", "0")))
    res = run_bass_kernel_spmd(
        nc, in_maps, core_ids=list(range(NCORES)), trace=trace
    )
    LAST_RESULTS = res

    wd_sum = 0.0
    det = 0.0
    for r in res.results:
        o = r["out"].astype(np.float64)
        wd_sum += o[:, 0:8].sum()
        det += o[:, 8:16].sum()

    if det != 0.0:
        # some weighted pair has d < beta+margin: shortcut invalid -> exact
        return _host_reference(batch, labels, triplets, beta)

    pos_sum = wd_sum - ht["cposw"]
    total = pos_sum + ht["sum"]
    cnt = ht["wptot"] + ht["cnt"]
    loss = total if cnt == 0.0 else total / cnt
    return np.float32(loss)


# revision 6
# speedup vs baseline: 1.1822x; 1.1822x over previous
"""Margin-based triplet loss (nn_Criterion) for Trainium2, 8 NeuronCores.

Strategy: anchor-block sharding + margin-slack reduction.  Core c owns
anchor rows [512c, 512c+512).  The host buckets triplets by anchor block
into dense pair-count histograms (wp for anchor-positive pairs, wc =
wp + wn for the detector); the device computes squared distances densely
via fp8 DoubleRow matmuls (with the norm terms folded in as augmented
bf16 contraction rows), takes one sqrt per tile, and reduces just TWO
dense quantities per anchor row:

    wd  = sum_b wp[a,b] * d[a,b]          (weighted distance sum)
    det = sum_b wc[a,b] * [d[a,b] < cneg] (margin-violation detector)

If det == 0 everywhere (certified on-device), the exact loss reduces to

    pos_sum = wd_total - sum_a cpos(a) * wp_row(a)     (host, f64)
    neg_sum = 0,  pair_count = total wp count
    (+ exact host corrections for degenerate a==p / a==n triplets)

because every weighted pair then has d > cneg > cpos, making every
pos relu affine-active and every neg relu inactive.  If det != 0 the
kernel falls back to an exact host computation (never triggers for
Gaussian-scale data; distances concentrate near sqrt(2D) >> beta+margin).
"""

import os

import numpy as np

B, D, T, NCLS = 4096, 1024, 65536, 100
MARGIN = 0.2
EPS = 1e-8
NCORES = 8
P = 128
RB = 512                 # anchor rows per core
NRB = RB // P            # 4 anchor row blocks
KCH = D // P             # 8 contraction chunks of 128
KP = KCH // 2            # 4 chunk pairs (DoubleRow)
GCOLS = 2048             # columns per join group
NJJ = B // GCOLS         # 2 column groups
NG = NRB * NJJ           # 8 groups per core
D2_BIAS = 0.0625         # keeps diagonal d^2 strictly positive for sqrt

_COMPILED = None
LAST_RESULTS = None


def _build_nc():
    import concourse.bacc as bacc
    import concourse.bass as bass
    import concourse.mybir as mybir
    import concourse.tile as tile

    f32 = mybir.dt.float32
    bf16 = mybir.dt.bfloat16
    f8 = mybir.dt.float8e4
    Alu = mybir.AluOpType
    Act = mybir.ActivationFunctionType

    use_dr = not bool(int(os.environ.get("KERNEL_NO_DR", "0")))
    dr = mybir.MatmulPerfMode.DoubleRow if use_dr else None

    nc = bacc.Bacc("TRN2")

    xt_d = nc.dram_tensor("xt", [P, KCH, B], f8, kind="ExternalInput")
    xat_d = nc.dram_tensor("xat", [P, KCH, RB], f8, kind="ExternalInput")
    wp_d = nc.dram_tensor("wp", [P, NRB, B], bf16, kind="ExternalInput")
    wc_d = nc.dram_tensor("wc", [P, NRB, B], bf16, kind="ExternalInput")
    augs_d = nc.dram_tensor("augs", [4, NRB, P], bf16, kind="ExternalInput")
    augm_d = nc.dram_tensor("augm", [4, B], bf16, kind="ExternalInput")
    cneg_d = nc.dram_tensor("cneg", [P, NRB], f32, kind="ExternalInput")
    out_d = nc.dram_tensor("out", [P, 16], f32, kind="ExternalOutput")

    with tile.TileContext(nc) as tc:
        with (
            tc.tile_pool(name="const", bufs=1) as constp,
            tc.tile_pool(name="xt", bufs=1) as xtp,
            tc.tile_pool(name="w", bufs=3) as wpool,
            tc.tile_pool(name="d", bufs=2) as dpool,
            tc.tile_pool(name="ps", bufs=2, space="PSUM") as gpsum,
        ):
            xat_sb = constp.tile([P, KCH, RB], f8, tag="xat")
            augs_sb = constp.tile([4, NRB, P], bf16, tag="augs")
            augm_sb = constp.tile([4, B], bf16, tag="augm")
            cneg_sb = constp.tile([P, NRB], f32, tag="cneg")
            acc = constp.tile([P, 16], f32, tag="acc")
            # small tiles first so the aug matmuls can start ASAP
            nc.sync.dma_start(augs_sb[:], augs_d[:])
            nc.sync.dma_start(augm_sb[:], augm_d[:])
            nc.sync.dma_start(cneg_sb[:], cneg_d[:])
            nc.sync.dma_start(xat_sb[:], xat_d[:])

            xtp_sb = []
            for i2 in range(KP):
                t = xtp.tile([P, 2, B], f8, tag=f"xt{i2}", name=f"xt{i2}")
                nc.sync.dma_start(t[:], xt_d[:, 2 * i2 : 2 * i2 + 2, :])
                xtp_sb.append(t)

            for r in range(NRB):
                for jj in range(NJJ):
                    gi = r * NJJ + jj
                    c0 = jj * GCOLS
                    wpt = wpool.tile([P, GCOLS], bf16, tag="wp")
                    wct = wpool.tile([P, GCOLS], bf16, tag="wc")
                    nc.sync.dma_start(wpt[:], wp_d[:, r, c0 : c0 + GCOLS])
                    nc.sync.dma_start(wct[:], wc_d[:, r, c0 : c0 + GCOLS])

                    g = gpsum.tile([P, GCOLS], f32, tag="g", space="PSUM")
                    # norm terms first (k=4 aug rows: na_hi, na_lo, 1, 1)
                    for j in range(GCOLS // 512):
                        cs = slice(c0 + j * 512, c0 + (j + 1) * 512)
                        nc.tensor.matmul(
                            g[:, j * 512 : (j + 1) * 512],
                            augs_sb[:, r, :],
                            augm_sb[:, cs],
                            start=True,
                            stop=False,
                        )
                    # -2 * Gram, fp8 (DoubleRow: two 128-chunks per pass)
                    if use_dr:
                        for i2 in range(KP):
                            lhs = xat_sb[:, 2 * i2 : 2 * i2 + 2, r * P : (r + 1) * P]
                            for j in range(GCOLS // 512):
                                cs = slice(c0 + j * 512, c0 + (j + 1) * 512)
                                nc.tensor.matmul(
                                    g[:, j * 512 : (j + 1) * 512],
                                    lhs,
                                    xtp_sb[i2][:, :, cs],
                                    start=False,
                                    stop=(i2 == KP - 1),
                                    perf_mode=dr,
                                )
                    else:
                        for i in range(KCH):
                            lhs = xat_sb[:, i, r * P : (r + 1) * P]
                            for j in range(GCOLS // 512):
                                cs = slice(c0 + j * 512, c0 + (j + 1) * 512)
                                nc.tensor.matmul(
                                    g[:, j * 512 : (j + 1) * 512],
                                    lhs,
                                    xtp_sb[i // 2][:, i % 2, cs],
                                    start=False,
                                    stop=(i == KCH - 1),
                                )

                    d = dpool.tile([P, GCOLS], bf16, tag="d")
                    nc.scalar.activation(d[:], g[:], Act.Sqrt)
                    dum_a = dpool.tile([P, GCOLS], bf16, tag="dum_a")
                    dum_b = dpool.tile([P, GCOLS], bf16, tag="dum_b")
                    nc.vector.scalar_tensor_tensor(
                        dum_a[:], d[:], 1.0, wpt[:],
                        Alu.mult, Alu.mult,
                        accum_out=acc[:, gi : gi + 1],
                    )
                    nc.vector.scalar_tensor_tensor(
                        dum_b[:], d[:], cneg_sb[:, r : r + 1], wct[:],
                        Alu.is_lt, Alu.mult,
                        accum_out=acc[:, 8 + gi : 8 + gi + 1],
                    )

            nc.sync.dma_start(out_d[:], acc[:])

    nc.compile()
    return nc


def _prep_inputs(batch, labels, triplets, beta):
    import ml_dtypes

    bf = ml_dtypes.bfloat16
    f8 = ml_dtypes.float8_e4m3
    trip = np.asarray(triplets).astype(np.int64)
    labs = np.asarray(labels).astype(np.int64)
    batch = np.ascontiguousarray(np.asarray(batch), dtype=np.float32)
    beta64 = np.asarray(beta).astype(np.float64)

    x8 = batch.astype(f8)
    x8f = x8.astype(np.float32)
    # norms of the fp8-rounded rows (consistency: d^2 = |a-b|^2 >= 0)
    n64 = (x8f.astype(np.float64) ** 2).sum(-1)
    n32 = n64.astype(np.float32)
    n_hi = n32.astype(bf)
    n_hi_f = n_hi.astype(np.float32)
    n_lo = (n32 - n_hi_f + np.float32(D2_BIAS)).astype(bf)

    # X^T chunked: xt[p, i, b] = x8[b, i*128+p]
    xt = np.ascontiguousarray(x8.reshape(B, KCH, P).transpose(2, 1, 0))
    # moving aug rows: [1, 1, nb_hi, nb_lo]
    augm = np.empty((4, B), dtype=bf)
    augm[0, :] = bf(1.0)
    augm[1, :] = bf(1.0)
    augm[2, :] = n_hi
    augm[3, :] = n_lo

    cpos_all = beta64[labs] - MARGIN          # [B] per-anchor pos threshold
    cneg_all = (beta64[labs] + MARGIN).astype(np.float32)

    t0, t1, t2 = trip[:, 0], trip[:, 1], trip[:, 2]
    deg_p = t0 == t1
    deg_n = t0 == t2

    # exact host corrections for degenerate triplets (d = sqrt(EPS))
    d_deg = np.sqrt(EPS)
    host_sum = 0.0
    host_cnt = 0.0
    if deg_p.any():
        pl = np.maximum(d_deg - beta64[labs[t0[deg_p]]] + MARGIN, 0.0)
        host_sum += pl.sum()
        host_cnt += (pl > 0).sum()
    if deg_n.any():
        nl = np.maximum(beta64[labs[t0[deg_n]]] + MARGIN - d_deg, 0.0)
        host_sum += nl.sum()
        host_cnt += (nl > 0).sum()

    in_maps = []
    host_terms = {"sum": host_sum, "cnt": host_cnt, "cposw": 0.0, "wptot": 0.0}
    xm2_all = (-2.0 * x8f)  # exact scaling of fp8 values
    for c in range(NCORES):
        lo, hi = c * RB, (c + 1) * RB
        sel = (t0 >= lo) & (t0 < hi)
        sp = sel & ~deg_p
        sn = sel & ~deg_n
        a_p = t0[sp] - lo
        a_n = t0[sn] - lo
        wp = np.bincount(a_p * B + t1[sp], minlength=RB * B).reshape(RB, B)
        wn = np.bincount(a_n * B + t2[sn], minlength=RB * B).reshape(RB, B)
        wc = wp + wn

        host_terms["cposw"] += float((cpos_all[lo:hi] * wp.sum(-1)).sum())
        host_terms["wptot"] += float(wp.sum())

        def togrid(w):
            return np.ascontiguousarray(
                w.reshape(NRB, P, B).transpose(1, 0, 2)
            ).astype(bf)

        xat = np.ascontiguousarray(
            xm2_all[lo:hi].astype(f8).reshape(RB, KCH, P).transpose(2, 1, 0)
        )
        augs = np.empty((4, NRB, P), dtype=bf)
        augs[0] = n_hi[lo:hi].reshape(NRB, P)
        augs[1] = n_lo[lo:hi].reshape(NRB, P)
        augs[2] = bf(1.0)
        augs[3] = bf(1.0)
        cneg = np.ascontiguousarray(cneg_all[lo:hi].reshape(NRB, P).T)

        in_maps.append(
            {
                "xt": xt,
                "xat": xat,
                "wp": togrid(wp),
                "wc": togrid(wc),
                "augs": augs,
                "augm": augm,
                "cneg": cneg,
            }
        )
    return in_maps, host_terms


def _host_reference(batch, labels, triplets, beta):
    """Exact fallback (f64 accumulation), chunked to bound memory."""
    batch = np.asarray(batch, dtype=np.float32)
    trip = np.asarray(triplets).astype(np.int64)
    labs = np.asarray(labels).astype(np.int64)
    beta64 = np.asarray(beta).astype(np.float64)
    total = 0.0
    cnt = 0.0
    for s in range(0, trip.shape[0], 8192):
        t = trip[s : s + 8192]
        a = batch[t[:, 0]].astype(np.float64)
        p = batch[t[:, 1]].astype(np.float64)
        n = batch[t[:, 2]].astype(np.float64)
        d_ap = np.sqrt(((a - p) ** 2).sum(-1) + EPS)
        d_an = np.sqrt(((a - n) ** 2).sum(-1) + EPS)
        bt = beta64[labs[t[:, 0]]]
        pos = np.maximum(d_ap - bt + MARGIN, 0.0)
        neg = np.maximum(bt - d_an + MARGIN, 0.0)
        total += pos.sum() + neg.sum()
        cnt += (pos > 0).sum() + (neg > 0).sum()
    return np.float32(total if cnt == 0.0 else total / cnt)


def kernel(batch, labels, triplets, beta):
    global _COMPILED, LAST_RESULTS
    from concourse.bass_utils import run_bass_kernel_spmd

    if _COMPILED is None:
        _COMPILED = _build_nc()
    nc = _COMPILED

    in_maps, ht = _prep_inputs(batch, labels, triplets, beta)
    trace = bool(int(os.environ.get("KERNEL_TRACE", "0")))
    res = run_bass_kernel_spmd(
        nc, in_maps, core_ids=list(range(NCORES)), trace=trace
    )
    LAST_RESULTS = res

    wd_sum = 0.0
    det = 0.0
    for r in res.results:
        o = r["out"].astype(np.float64)
        wd_sum += o[:, 0:8].sum()
        det += o[:, 8:16].sum()

    if det != 0.0:
        # some weighted pair has d < beta+margin: shortcut invalid -> exact
        return _host_reference(batch, labels, triplets, beta)

    pos_sum = wd_sum - ht["cposw"]
    total = pos_sum + ht["sum"]
    cnt = ht["wptot"] + ht["cnt"]
    loss = total if cnt == 0.0 else total / cnt
    return np.float32(loss)
